# revision 4
# baseline (speedup 1.0000x reference)
"""ContrastiveDist kernel for TRN2 (8 NeuronCores, SPMD).

out[n] = sum_e -(t_e . v_n) / (||t_e|| * ||v_n|| + eps)
       = -(s . v_n) / ||v_n||          with s = sum_e t_e / ||t_e||
(eps shifts the result by ~eps/(||t||*||v||) ~ 4e-11 relative -- far
below fp32 noise, so it is dropped.)

Sharding: node_emb split across 8 cores (6250 rows each, padded to
6272 = 49*128); target replicated.  Per-core layout puts node n at
(partition p, tile t) with n = p*49 + t, so tile-windows are contiguous
in DRAM per partition (8KB DMA packets) and the final [128, 49] result
stores with one partition-contiguous DMA.

Phases (emit order = engine FIFO order, chosen so no engine stalls):
  1. target DMA [128,16,256] (entity e = p*16+j, 16KB/partition packets)
     + 7 node chunk DMAs [128,<=8,256].
  2. phase A on target: ACT square -> DVE reduce -> ACT sqrt -> DVE
     reciprocal -> 16 fp32r matmuls (lhsT = 1/||t|| column, rhs = target
     tile; fp32r is 4x fp32 at N=256) accumulating s in PSUM -> ACT copy
     -> GpSimd partition_broadcast -> s_b [128,256].
  3. node ssq: first ACT_CHUNKS chunks per-tile on ACT (Square with
     fused accum_out), rest as batched ACT square + DVE 3D-AP reduce --
     splits the work so DVE and ACT finish together.
  4. dots (after s_b): batched DVE mul (stride-0 broadcast of s_b) +
     reduce(negate) per chunk.
  5. tail: sqrt, reciprocal, multiply, one 25KB store.
"""

import numpy as np
from contextlib import ExitStack

import concourse.bacc as bacc
import concourse.bass as bass
import concourse.mybir as mybir
import concourse.tile as tile
from concourse import bass_utils

E, D = 2048, 256          # entities, embed dim
N_FULL = 50000            # total nodes
N_CORES = 8
NPC = N_FULL // N_CORES   # 6250 true nodes per core
TPC = 49                  # node tiles per core (49*128 = 6272 padded)
NPAD = TPC * 128
ET = E // 128             # 16 entity tiles

CHUNKS = [8, 8, 8, 8, 8, 8, 1]          # node tiles per DMA/compute chunk
ACT_CHUNKS = 3                          # leading chunks: ssq per-tile on ACT

F32 = mybir.dt.float32
F32R = mybir.dt.float32r

_cache = {}


def _build():
    nc = bacc.Bacc(
        "TRN2",
        target_bir_lowering=False,
        debug=False,
        enable_asserts=True,
        num_devices=N_CORES,
    )
    tgt = nc.dram_tensor("target", [E, D], F32, kind="ExternalInput").ap()
    nodes = nc.dram_tensor("nodes", [NPAD, D], F32, kind="ExternalInput").ap()
    out = nc.dram_tensor("out", [NPAD], F32, kind="ExternalOutput").ap()

    with tile.TileContext(nc) as tc, ExitStack() as ctx:
        tpool = ctx.enter_context(tc.tile_pool(name="tgt", bufs=1))
        vpool = ctx.enter_context(tc.tile_pool(name="v", bufs=1))
        spool = ctx.enter_context(tc.tile_pool(name="small", bufs=1))
        scr_pool = ctx.enter_context(tc.tile_pool(name="scr", bufs=2))
        scr2_pool = ctx.enter_context(tc.tile_pool(name="scr2", bufs=2))
        scrA_pool = ctx.enter_context(tc.tile_pool(name="scrA", bufs=2))
        psum = ctx.enter_context(tc.tile_pool(name="psum", bufs=1, space="PSUM"))

        # ---- DMAs first: target, then node chunks (issue order = queue order)
        tgt_sb = tpool.tile([128, ET, D], F32)
        nc.sync.dma_start(tgt_sb[:], tgt.rearrange("(p j) d -> p j d", j=ET))

        nodes_v = nodes.rearrange("(p t) d -> p t d", t=TPC)
        v_tiles = []
        off = 0
        for c, w in enumerate(CHUNKS):
            v = vpool.tile([128, w, D], F32, tag=f"v{c}")
            nc.sync.dma_start(v[:], nodes_v[:, off : off + w, :])
            v_tiles.append((v, off, w))
            off += w

        # ---- phase A: s = sum_e target[e] / ||target[e]|| (entities permuted
        # e = p*16 + j; the sum is permutation invariant)
        scrT = tpool.tile([128, ET, D], F32, tag="scrT")
        nc.scalar.activation(
            scrT[:], tgt_sb[:], mybir.ActivationFunctionType.Square
        )
        ssq_t = spool.tile([128, ET], F32)
        nc.vector.tensor_reduce(
            ssq_t[:], scrT[:], axis=mybir.AxisListType.X, op=mybir.AluOpType.add
        )
        tn = spool.tile([128, ET], F32)
        nc.scalar.sqrt(tn[:], ssq_t[:])
        inv_tn = spool.tile([128, ET], F32)
        nc.vector.reciprocal(inv_tn[:], tn[:])

        ps = psum.tile([1, D], F32)
        for j in range(ET):
            nc.tensor.matmul(
                ps[:],
                inv_tn[:, j : j + 1],
                tgt_sb[:, j, :],
                start=(j == 0),
                stop=(j == ET - 1),
            )
        s_row = spool.tile([1, D], F32)
        nc.scalar.copy(s_row[:], ps[:])
        s_b = spool.tile([128, D], F32)
        nc.gpsimd.partition_broadcast(s_b[:], s_row[:])

        # ---- node ssq (independent of s_b; starts as soon as data lands)
        ssq_v = spool.tile([128, TPC], F32)
        for c, (v, off, w) in enumerate(v_tiles):
            if c < ACT_CHUNKS:
                # per-tile Square with fused accumulate on ACT
                for j in range(w):
                    scrA = scrA_pool.tile([128, D], F32)
                    nc.scalar.activation(
                        scrA[:], v[:, j, :], mybir.ActivationFunctionType.Square,
                        accum_out=ssq_v[:, off + j : off + j + 1],
                    )
            else:
                scr2 = scr2_pool.tile([128, w, D], F32)
                nc.scalar.activation(
                    scr2[:], v[:], mybir.ActivationFunctionType.Square
                )
                nc.vector.tensor_reduce(
                    ssq_v[:, off : off + w], scr2[:],
                    axis=mybir.AxisListType.X, op=mybir.AluOpType.add,
                )

        # ---- dots: -v.s per node (DVE, batched; s_b broadcast at stride 0)
        negdot = spool.tile([128, TPC], F32)
        for c, (v, off, w) in enumerate(v_tiles):
            scr = scr_pool.tile([128, w, D], F32)
            nc.vector.tensor_mul(
                scr[:], v[:], s_b[:].unsqueeze(1).broadcast_to([128, w, D])
            )
            nc.vector.tensor_reduce(
                negdot[:, off : off + w], scr[:],
                axis=mybir.AxisListType.X, op=mybir.AluOpType.add, negate=True,
            )

        # ---- tail
        vn = spool.tile([128, TPC], F32)
        nc.scalar.sqrt(vn[:], ssq_v[:])
        inv_vn = spool.tile([128, TPC], F32)
        nc.vector.reciprocal(inv_vn[:], vn[:])
        res = spool.tile([128, TPC], F32)
        nc.vector.tensor_mul(res[:], negdot[:], inv_vn[:])
        nc.sync.dma_start(out.rearrange("(p t) -> p t", t=TPC), res[:])

    nc.compile()
    return nc


def _get_nc():
    if "nc" not in _cache:
        _cache["nc"] = _build()
    return _cache["nc"]


def run(pred, target, node_emb, trace=False, **trace_kwargs):
    """Returns (full_output [50000] f32, BassKernelResults)."""
    target = np.ascontiguousarray(np.asarray(target, dtype=np.float32))
    node_emb = np.ascontiguousarray(np.asarray(node_emb, dtype=np.float32))

    nc = _get_nc()
    in_maps = []
    for c in range(N_CORES):
        shard = np.empty((NPAD, D), dtype=np.float32)
        shard[:NPC] = node_emb[c * NPC : (c + 1) * NPC]
        shard[NPC:] = node_emb[: NPAD - NPC]  # pad with real rows (no 0-norm)
        in_maps.append({"target": target, "nodes": shard})

    res = bass_utils.run_bass_kernel_spmd(
        nc, in_maps, list(range(N_CORES)), trace=trace, **trace_kwargs
    )
    parts = [res.results[c]["out"][:NPC] for c in range(N_CORES)]
    return np.concatenate(parts).astype(np.float32), res


def kernel(pred, target, node_emb):
    out, _ = run(pred, target, node_emb)
    return out


# revision 6
# speedup vs baseline: 1.1246x; 1.1246x over previous
"""ContrastiveDist kernel for TRN2 (8 NeuronCores, SPMD).

out[n] = sum_e -(t_e . v_n) / (||t_e|| * ||v_n|| + eps)
       = -(s . v_n) / ||v_n||          with s = sum_e t_e / ||t_e||
(eps shifts the result by ~eps/(||t||*||v||) ~ 4e-11 relative -- far
below fp32 noise, so it is dropped.)

Sharding: node_emb split across 8 cores (6250 rows each, padded to
6272 = 49*128); target replicated.  Per-core layout puts node n at
(partition p, tile t) with n = p*49 + t, so tile-windows are contiguous
in DRAM per partition (8KB DMA packets) and the final [128, 49] result
stores with one partition-contiguous DMA.

Phases (emit order = engine FIFO order, chosen so no engine stalls):
  1. target DMA [128,16,256] (entity e = p*16+j, 16KB/partition packets)
     + 7 node chunk DMAs [128,<=8,256].
  2. phase A on target: ACT square -> DVE reduce -> ACT sqrt -> DVE
     reciprocal -> 16 fp32r matmuls (lhsT = 1/||t|| column, rhs = target
     tile; fp32r is 4x fp32 at N=256) accumulating s in PSUM -> ACT copy
     -> GpSimd partition_broadcast -> s_b [128,256].
  3. node ssq: first ACT_CHUNKS chunks per-tile on ACT (Square with
     fused accum_out), rest as batched ACT square + DVE 3D-AP reduce --
     splits the work so DVE and ACT finish together.
  4. dots (after s_b): batched DVE mul (stride-0 broadcast of s_b) +
     reduce(negate) per chunk.
  5. tail: sqrt, reciprocal, multiply, one 25KB store.
"""

import numpy as np
from contextlib import ExitStack

import concourse.bacc as bacc
import concourse.bass as bass
import concourse.mybir as mybir
import concourse.tile as tile
from concourse import bass_utils

E, D = 2048, 256          # entities, embed dim
N_FULL = 50000            # total nodes
N_CORES = 8
NPC = N_FULL // N_CORES   # 6250 true nodes per core
TPC = 49                  # node tiles per core (49*128 = 6272 padded)
NPAD = TPC * 128
ET = E // 128             # 16 entity tiles

CHUNKS = [8, 8, 8, 8, 8, 8, 1]          # node tiles per DMA/compute chunk
ACT_CHUNKS = 3                          # leading chunks: ssq per-tile on ACT

F32 = mybir.dt.float32
F32R = mybir.dt.float32r

_cache = {}


def _build():
    nc = bacc.Bacc(
        "TRN2",
        target_bir_lowering=False,
        debug=False,
        enable_asserts=True,
        num_devices=N_CORES,
    )
    tgt = nc.dram_tensor("target", [E, D], F32, kind="ExternalInput").ap()
    nodes = nc.dram_tensor("nodes", [NPAD, D], F32, kind="ExternalInput").ap()
    out = nc.dram_tensor("out", [NPAD], F32, kind="ExternalOutput").ap()

    with tile.TileContext(nc) as tc, ExitStack() as ctx:
        tpool = ctx.enter_context(tc.tile_pool(name="tgt", bufs=1))
        vpool = ctx.enter_context(tc.tile_pool(name="v", bufs=1))
        spool = ctx.enter_context(tc.tile_pool(name="small", bufs=1))
        scr_pool = ctx.enter_context(tc.tile_pool(name="scr", bufs=2))
        scr2_pool = ctx.enter_context(tc.tile_pool(name="scr2", bufs=2))
        scrA_pool = ctx.enter_context(tc.tile_pool(name="scrA", bufs=2))
        psum = ctx.enter_context(tc.tile_pool(name="psum", bufs=1, space="PSUM"))

        # ---- DMAs first: target (4 pipelined chunks), then node chunks
        TC = 4  # target pipeline chunks of 4 entity-tiles each
        tgt_sb = tpool.tile([128, ET, D], F32)
        tgt_v = tgt.rearrange("(p j) d -> p j d", j=ET)
        for k in range(TC):
            sl = slice(k * (ET // TC), (k + 1) * (ET // TC))
            nc.sync.dma_start(tgt_sb[:, sl, :], tgt_v[:, sl, :])

        nodes_v = nodes.rearrange("(p t) d -> p t d", t=TPC)
        v_tiles = []
        off = 0
        for c, w in enumerate(CHUNKS):
            v = vpool.tile([128, w, D], F32, tag=f"v{c}")
            nc.sync.dma_start(v[:], nodes_v[:, off : off + w, :])
            v_tiles.append((v, off, w))
            off += w

        # ---- phase A: s = sum_e target[e] / ||target[e]|| (entities permuted
        # e = p*16 + j; the sum is permutation invariant).  Fully pipelined
        # per target chunk so the PE matmuls start as early as possible.
        ssq_t = spool.tile([128, ET], F32)
        inv_tn = spool.tile([128, ET], F32)
        ps = psum.tile([1, D], F32)
        W = ET // TC
        for k in range(TC):
            sl = slice(k * W, (k + 1) * W)
            scrT = scr2_pool.tile([128, W, D], F32, tag="scrT")
            nc.scalar.activation(
                scrT[:], tgt_sb[:, sl, :], mybir.ActivationFunctionType.Square
            )
            nc.vector.tensor_reduce(
                ssq_t[:, sl], scrT[:],
                axis=mybir.AxisListType.X, op=mybir.AluOpType.add,
            )
            tn_k = spool.tile([128, W], F32, tag=f"tn{k}")
            nc.scalar.sqrt(tn_k[:], ssq_t[:, sl])
            nc.vector.reciprocal(inv_tn[:, sl], tn_k[:])
            for j in range(k * W, (k + 1) * W):
                nc.tensor.matmul(
                    ps[:],
                    inv_tn[:, j : j + 1],
                    tgt_sb[:, j, :],
                    start=(j == 0),
                    stop=(j == ET - 1),
                )
        s_row = spool.tile([1, D], F32)
        nc.scalar.copy(s_row[:], ps[:])
        s_b = spool.tile([128, D], F32)
        nc.gpsimd.partition_broadcast(s_b[:], s_row[:])

        # ---- node ssq (independent of s_b; starts as soon as data lands)
        ssq_v = spool.tile([128, TPC], F32)
        for c, (v, off, w) in enumerate(v_tiles):
            if c < ACT_CHUNKS:
                # per-tile Square with fused accumulate on ACT
                for j in range(w):
                    scrA = scrA_pool.tile([128, D], F32)
                    nc.scalar.activation(
                        scrA[:], v[:, j, :], mybir.ActivationFunctionType.Square,
                        accum_out=ssq_v[:, off + j : off + j + 1],
                    )
            else:
                scr2 = scr2_pool.tile([128, w, D], F32)
                nc.scalar.activation(
                    scr2[:], v[:], mybir.ActivationFunctionType.Square
                )
                nc.vector.tensor_reduce(
                    ssq_v[:, off : off + w], scr2[:],
                    axis=mybir.AxisListType.X, op=mybir.AluOpType.add,
                )

        # ---- dots: -v.s per node (DVE, batched; s_b broadcast at stride 0)
        negdot = spool.tile([128, TPC], F32)
        for c, (v, off, w) in enumerate(v_tiles):
            scr = scr_pool.tile([128, w, D], F32)
            nc.vector.tensor_mul(
                scr[:], v[:], s_b[:].unsqueeze(1).broadcast_to([128, w, D])
            )
            nc.vector.tensor_reduce(
                negdot[:, off : off + w], scr[:],
                axis=mybir.AxisListType.X, op=mybir.AluOpType.add, negate=True,
            )

        # ---- tail
        vn = spool.tile([128, TPC], F32)
        nc.scalar.sqrt(vn[:], ssq_v[:])
        inv_vn = spool.tile([128, TPC], F32)
        nc.vector.reciprocal(inv_vn[:], vn[:])
        res = spool.tile([128, TPC], F32)
        nc.vector.tensor_mul(res[:], negdot[:], inv_vn[:])
        nc.sync.dma_start(out.rearrange("(p t) -> p t", t=TPC), res[:])

    nc.compile()
    return nc


def _get_nc():
    if "nc" not in _cache:
        _cache["nc"] = _build()
    return _cache["nc"]


def run(pred, target, node_emb, trace=False, **trace_kwargs):
    """Returns (full_output [50000] f32, BassKernelResults)."""
    target = np.ascontiguousarray(np.asarray(target, dtype=np.float32))
    node_emb = np.ascontiguousarray(np.asarray(node_emb, dtype=np.float32))

    nc = _get_nc()
    in_maps = []
    for c in range(N_CORES):
        shard = np.empty((NPAD, D), dtype=np.float32)
        shard[:NPC] = node_emb[c * NPC : (c + 1) * NPC]
        shard[NPC:] = node_emb[: NPAD - NPC]  # pad with real rows (no 0-norm)
        in_maps.append({"target": target, "nodes": shard})

    res = bass_utils.run_bass_kernel_spmd(
        nc, in_maps, list(range(N_CORES)), trace=trace, **trace_kwargs
    )
    parts = [res.results[c]["out"][:NPC] for c in range(N_CORES)]
    return np.concatenate(parts).astype(np.float32), res


def kernel(pred, target, node_emb):
    out, _ = run(pred, target, node_emb)
    return out
